# revision 21
# baseline (speedup 1.0000x reference)
"""Trainium2 Bass kernel for nn_CustomSegmentationLayer (retrieval_knn).

Pipeline per image (reference semantics):
  - sample 50 fg + 50 bg training pixels via jax RNG (host, bit-exact
    vmapped replication of the reference's sampling)
  - 5-D pixel features (3 color + 2 weighted position), standardized by
    train mean/std
  - brute-force 5-NN over the 100 train pixels, fg-vote >= 2/5 -> seg mask
  - output = preprocessed image masked by seg

Device formulation: for pixel p and train candidate j,
    m[p, j] = 2*t_p.s_j - ||s_j||^2   (t = standardized test feat,
                                       s = standardized train feat)
is a monotone (reversed) proxy for squared distance, computed as one
K=6 matmul per 128-pixel tile:  feats[6,128].T @ W[6,100], where the
host folds standardization, the 2x, the /255^2 color scale and the
||s||^2 bias (via a ones-row) into W.

Vote rule: seg_p = 1  iff  2nd-smallest fg distance <= 4th-smallest bg
distance, i.e. fgmax8[:,1] >= bgmax8[:,3] on m (ties favor fg exactly
like the reference's index-ordered top_k). One vector.max (top-8 per
partition) per 50-candidate half per tile.

Sharding: pure data parallel, 4 images per core on 8 cores.
"""

import numpy as np

H = W = 96
C = 3
N_PIX = H * W          # 9216
NPC = 50               # train samples per class
N_CAND = 2 * NPC       # 100
B = 32                 # batch
N_CORES = 8
IPC = B // N_CORES     # images per core = 4
NT = N_PIX // 128      # 72 pixel tiles per image
GROUP = 5              # score tiles per PSUM bank (5*100 <= 512)
POS_WEIGHT = 100.0
COLOR_SCALE = 255.0 * 255.0  # raw -> feature scale for color dims

_CACHE = {}


def _host_sampling(images_np):
    """Replicate the reference's vmapped sampling + standardization on CPU.

    Must mirror the reference *under vmap* — jax's batched RNG produces
    different streams than per-image calls.
    Returns train_s [B,100,5], mean [B,5], std [B,5] (fp32, bit-exact
    w.r.t. the reference).
    """
    import jax
    import jax.numpy as jnp
    from jax import lax

    cpu = jax.devices("cpu")[0]

    def sample_part(img, key):
        img_p = jnp.clip(img / 255.0, 0.0, 1.0)
        img_norm = img_p / jnp.max(img_p, axis=(0, 1), keepdims=True)
        cleaned = img_norm * (img_norm < 0.6).astype(jnp.float32)
        fg = jnp.any(cleaned > 0, axis=-1).reshape(-1)
        bg = ~fg

        k_fg, k_bg = jax.random.split(key)

        def samp(k, mask):
            u = jax.random.uniform(k, (N_PIX,))
            score = jnp.where(mask, u, -1.0)
            _, idx = lax.top_k(score, NPC)
            return idx

        fg_idx = samp(k_fg, fg)
        bg_idx = samp(k_bg, bg)

        ii, jj = jnp.meshgrid(jnp.arange(H), jnp.arange(W), indexing="ij")
        pos = jnp.stack([ii, jj], axis=-1).reshape(-1, 2).astype(jnp.float32)
        pos = pos / jnp.array([H, W], jnp.float32) * POS_WEIGHT
        feats_all = jnp.concatenate([img_p.reshape(-1, C) / 255.0, pos], axis=1)
        train = jnp.concatenate([feats_all[fg_idx], feats_all[bg_idx]], axis=0)
        mean = jnp.mean(train, axis=0)
        std = jnp.std(train, axis=0)
        train_s = (train - mean) / std
        return train_s, mean, std

    with jax.default_device(cpu):
        keys = jax.random.split(jax.random.key(42), B)
        train_s, mean, std = jax.vmap(sample_part)(jnp.asarray(images_np), keys)
        return (np.asarray(train_s), np.asarray(mean), np.asarray(std))


def _build_weights(train_s, mean, std):
    """Fold standardization + bias into W [B, 6, 100] fp32 (f64 math)."""
    s = train_s.astype(np.float64)          # [B,100,5]
    mn = mean.astype(np.float64)            # [B,5]
    sd = std.astype(np.float64)             # [B,5]
    a = s / sd[:, None, :]                  # [B,100,5]
    Wm = np.empty((B, 6, N_CAND), np.float64)
    Wm[:, 0:3, :] = (2.0 * a[:, :, 0:3] / COLOR_SCALE).transpose(0, 2, 1)
    Wm[:, 3:5, :] = (2.0 * a[:, :, 3:5]).transpose(0, 2, 1)
    Wm[:, 5, :] = -np.sum(s * s, axis=2) - 2.0 * np.sum(mn[:, None, :] * a, axis=2)
    return Wm.astype(np.float32)


def _pos_features():
    ii, jj = np.meshgrid(np.arange(H), np.arange(W), indexing="ij")
    pos = np.stack([ii, jj], -1).reshape(-1, 2).astype(np.float32)
    return pos / np.array([H, W], np.float32) * np.float32(POS_WEIGHT)  # [N_PIX,2]


def _build_bass():
    import concourse.bacc as bacc
    import concourse.mybir as mybir
    from concourse import tile

    f32 = mybir.dt.float32
    f16 = mybir.dt.float16
    nc = bacc.Bacc("TRN2", target_bir_lowering=False, debug=False)

    # fp16 hi/lo split folded into ONE K=24 matmul per tile:
    #   lhsT rows = [fh(6); fl(6); fh(6); fl(6)]
    #   rhs  rows = [Wh(6); Wh(6); Wl(6); Wl(6)]
    # so one MM accumulates fh.Wh + fl.Wh + fh.Wl + fl.Wl — the full
    # fp32-accurate product. MM time is N cycles regardless of K.
    f24_d = nc.dram_tensor("feats24", [IPC, 24, N_PIX], f16, kind="ExternalInput")
    w24_d = nc.dram_tensor("w24", [24, IPC, N_CAND], f16, kind="ExternalInput")
    imgpm_d = nc.dram_tensor("imgpm", [IPC, 128, NT * C], f32, kind="ExternalInput")
    out_d = nc.dram_tensor("out", [IPC, 128, NT * C], f32, kind="ExternalOutput")

    n_groups = (NT + GROUP - 1) // GROUP

    with tile.TileContext(nc) as tc:
        with (
            tc.tile_pool(name="const", bufs=1) as cpool,
            tc.tile_pool(name="sb", bufs=2) as sb,
            tc.tile_pool(name="scores", bufs=4) as scp,
            tc.tile_pool(name="psum", bufs=8, space="PSUM") as pp,
        ):
            w24_sb = cpool.tile([24, IPC, N_CAND], f16)
            nc.sync.dma_start(out=w24_sb[:], in_=w24_d[:])

            # feats arrive in chunk-tiles (2/6/7 groups) so group 0's matmuls
            # start as soon as the first ~60KB land; imgpm/out ride the
            # gpsimd DMA queue to keep the sync queue purely feats.
            bounds = [0, 2 * GROUP, 8 * GROUP, 15 * GROUP]  # tiles [0,10,40,75]
            for i in range(IPC):
                f24_ch = []
                for ci in range(3):
                    t0, t1 = bounds[ci], min(bounds[ci + 1], NT)
                    ch = sb.tile(
                        [24, (t1 - t0) * 128], f16, tag=f"f24_{ci}", name=f"f24_{ci}"
                    )
                    nc.sync.dma_start(
                        out=ch[:], in_=f24_d[i, :, t0 * 128 : t1 * 128]
                    )
                    f24_ch.append(ch)
                imgpm_sb = sb.tile([128, NT * C], f32, tag="imgpm")
                nc.gpsimd.dma_start(out=imgpm_sb[:], in_=imgpm_d[i])

                # seg/mask runs in two halves (tiles 0..39 and 40..71) so the
                # first half's epilogue overlaps the second half's max8 stream;
                # split tiles keep the dependencies precise.
                HALF_T = [0, 8 * GROUP, NT]  # [0, 40, 72]
                fgmax_h = []
                bgmax_h = []
                for h in range(2):
                    nt_h = HALF_T[h + 1] - HALF_T[h]
                    fgmax_h.append(
                        sb.tile([128, nt_h * 8], f32, tag=f"fgmax{h}", name=f"fgmax{h}")
                    )
                    bgmax_h.append(
                        sb.tile([128, nt_h * 8], f32, tag=f"bgmax{h}", name=f"bgmax{h}")
                    )

                def emit_seg_half(h):
                    t0, t1 = HALF_T[h], HALF_T[h + 1]
                    nt_h = t1 - t0
                    seg = sb.tile([128, nt_h], f32, tag=f"seg{h}", name=f"seg{h}")
                    fg_r = fgmax_h[h][:].rearrange("p (t k) -> p t k", k=8)
                    bg_r = bgmax_h[h][:].rearrange("p (t k) -> p t k", k=8)
                    nc.vector.tensor_tensor(
                        seg[:], fg_r[:, :, 1], bg_r[:, :, 3], mybir.AluOpType.is_ge
                    )
                    out_sb = sb.tile(
                        [128, nt_h * C], f32, tag=f"out{h}", name=f"out{h}"
                    )
                    img_r = imgpm_sb[:, t0 * C : t1 * C].rearrange(
                        "p (t c) -> p t c", c=C
                    )
                    out_r = out_sb[:].rearrange("p (t c) -> p t c", c=C)
                    nc.vector.tensor_tensor(
                        out_r, img_r,
                        seg[:, :, None].to_broadcast([128, nt_h, C]),
                        mybir.AluOpType.mult,
                    )
                    nc.gpsimd.dma_start(
                        out=out_d[i, :, t0 * C : t1 * C], in_=out_sb[:]
                    )

                for g in range(n_groups):
                    ntg = min(GROUP, NT - g * GROUP)
                    ps = pp.tile([128, GROUP * N_CAND], f32, tag="ps")
                    for q in range(ntg):
                        t = g * GROUP + q
                        ci = next(k for k in range(3) if t < bounds[k + 1])
                        off = (t - bounds[ci]) * 128
                        nc.tensor.matmul(
                            ps[:, q * N_CAND : (q + 1) * N_CAND],
                            f24_ch[ci][:, off : off + 128],
                            w24_sb[:, i, :],
                            start=True,
                            stop=True,
                        )
                    sc = scp.tile([128, GROUP * N_CAND], f32, tag="sc")
                    nc.scalar.copy(
                        out=sc[:, : ntg * N_CAND], in_=ps[:, : ntg * N_CAND]
                    )
                    for q in range(ntg):
                        t = g * GROUP + q
                        h = 0 if t < HALF_T[1] else 1
                        toff = t - HALF_T[h]
                        nc.vector.max(
                            out=fgmax_h[h][:, toff * 8 : (toff + 1) * 8],
                            in_=sc[:, q * N_CAND : q * N_CAND + NPC],
                        )
                        nc.vector.max(
                            out=bgmax_h[h][:, toff * 8 : (toff + 1) * 8],
                            in_=sc[:, q * N_CAND + NPC : (q + 1) * N_CAND],
                        )
                    if g == 7:
                        emit_seg_half(0)
                emit_seg_half(1)

    nc.compile()
    return nc


def _get_nc():
    if "nc" not in _CACHE:
        _CACHE["nc"] = _build_bass()
    return _CACHE["nc"]


def prepare_in_maps(images: np.ndarray) -> list:
    """Host preamble: sampling + weight folding + device data layouts."""
    images = np.asarray(images, dtype=np.float32)
    assert images.shape == (B, H, W, C)

    train_s, mean, std = _host_sampling(images)
    Wall = _build_weights(train_s, mean, std)        # [B,6,100]
    pos = _pos_features()                            # [N_PIX,2]

    flat = images.reshape(B, N_PIX, C)
    feats = np.empty((B, 6, N_PIX), np.float32)
    feats[:, 0:3, :] = flat.transpose(0, 2, 1)
    feats[:, 3:5, :] = pos.T[None]
    feats[:, 5, :] = 1.0
    f16h = feats.astype(np.float16)
    f16l = (feats - f16h.astype(np.float32)).astype(np.float16)
    w16h = Wall.astype(np.float16)
    w16l = (Wall - w16h.astype(np.float32)).astype(np.float16)
    # K=24 stacking: feats rows [fh; fl; fh; fl], W rows [Wh; Wh; Wl; Wl]
    f24 = np.concatenate([f16h, f16l, f16h, f16l], axis=1)          # [B,24,NPIX]
    w24 = np.concatenate([w16h, w16h, w16l, w16l], axis=1)          # [B,24,100]
    # pixel-major tiles of the PREPROCESSED image (img_p = img/255, which
    # equals the reference's clip(img/255,0,1) bit-exactly for inputs < 255):
    # imgpm[b, p, t*3+c] = img_p[b, t*128+p, c]. The device multiplies by the
    # 1.0/0.0 seg mask directly, so kept pixels are bit-exact.
    imgpm = np.ascontiguousarray(
        (flat / np.float32(255.0)).reshape(B, NT, 128, C).transpose(0, 2, 1, 3)
    ).reshape(B, 128, NT * C)

    in_maps = []
    for c in range(N_CORES):
        sl = slice(c * IPC, (c + 1) * IPC)
        in_maps.append(
            {
                "feats24": np.ascontiguousarray(f24[sl]),
                "w24": np.ascontiguousarray(w24[sl].transpose(1, 0, 2)),
                "imgpm": np.ascontiguousarray(imgpm[sl]),
            }
        )
    return in_maps


def assemble_output(results: list) -> np.ndarray:
    out = np.empty((B, N_PIX, C), np.float32)
    for c in range(N_CORES):
        o = results[c]["out"]  # [IPC, 128, NT*C]
        o = o.reshape(IPC, 128, NT, C).transpose(0, 2, 1, 3).reshape(IPC, N_PIX, C)
        out[c * IPC : (c + 1) * IPC] = o
    return out.reshape(B, H, W, C)


def kernel(images: np.ndarray) -> np.ndarray:
    from concourse.bass_utils import run_bass_kernel_spmd

    in_maps = prepare_in_maps(images)
    nc = _get_nc()
    res = run_bass_kernel_spmd(nc, in_maps, core_ids=list(range(N_CORES)))
    return assemble_output(res.results)


# revision 24
# speedup vs baseline: 1.0106x; 1.0106x over previous
"""Trainium2 Bass kernel for nn_CustomSegmentationLayer (retrieval_knn).

Pipeline per image (reference semantics):
  - sample 50 fg + 50 bg training pixels via jax RNG (host, bit-exact
    vmapped replication of the reference's sampling)
  - 5-D pixel features (3 color + 2 weighted position), standardized by
    train mean/std
  - brute-force 5-NN over the 100 train pixels, fg-vote >= 2/5 -> seg mask
  - output = preprocessed image masked by seg

Device formulation: for pixel p and train candidate j,
    m[p, j] = 2*t_p.s_j - ||s_j||^2   (t = standardized test feat,
                                       s = standardized train feat)
is a monotone (reversed) proxy for squared distance, computed as one
K=6 matmul per 128-pixel tile:  feats[6,128].T @ W[6,100], where the
host folds standardization, the 2x, the /255^2 color scale and the
||s||^2 bias (via a ones-row) into W.

Vote rule: seg_p = 1  iff  2nd-smallest fg distance <= 4th-smallest bg
distance, i.e. fgmax8[:,1] >= bgmax8[:,3] on m (ties favor fg exactly
like the reference's index-ordered top_k). One vector.max (top-8 per
partition) per 50-candidate half per tile.

Sharding: pure data parallel, 4 images per core on 8 cores.
"""

import numpy as np

H = W = 96
C = 3
N_PIX = H * W          # 9216
NPC = 50               # train samples per class
N_CAND = 2 * NPC       # 100
B = 32                 # batch
N_CORES = 8
IPC = B // N_CORES     # images per core = 4
NT = N_PIX // 128      # 72 pixel tiles per image
GROUP = 5              # score tiles per PSUM bank (5*100 <= 512)
POS_WEIGHT = 100.0
COLOR_SCALE = 255.0 * 255.0  # raw -> feature scale for color dims

_CACHE = {}


def _host_sampling(images_np):
    """Replicate the reference's vmapped sampling + standardization on CPU.

    Must mirror the reference *under vmap* — jax's batched RNG produces
    different streams than per-image calls.
    Returns train_s [B,100,5], mean [B,5], std [B,5] (fp32, bit-exact
    w.r.t. the reference).
    """
    import jax
    import jax.numpy as jnp
    from jax import lax

    cpu = jax.devices("cpu")[0]

    def sample_part(img, key):
        img_p = jnp.clip(img / 255.0, 0.0, 1.0)
        img_norm = img_p / jnp.max(img_p, axis=(0, 1), keepdims=True)
        cleaned = img_norm * (img_norm < 0.6).astype(jnp.float32)
        fg = jnp.any(cleaned > 0, axis=-1).reshape(-1)
        bg = ~fg

        k_fg, k_bg = jax.random.split(key)

        def samp(k, mask):
            u = jax.random.uniform(k, (N_PIX,))
            score = jnp.where(mask, u, -1.0)
            _, idx = lax.top_k(score, NPC)
            return idx

        fg_idx = samp(k_fg, fg)
        bg_idx = samp(k_bg, bg)

        ii, jj = jnp.meshgrid(jnp.arange(H), jnp.arange(W), indexing="ij")
        pos = jnp.stack([ii, jj], axis=-1).reshape(-1, 2).astype(jnp.float32)
        pos = pos / jnp.array([H, W], jnp.float32) * POS_WEIGHT
        feats_all = jnp.concatenate([img_p.reshape(-1, C) / 255.0, pos], axis=1)
        train = jnp.concatenate([feats_all[fg_idx], feats_all[bg_idx]], axis=0)
        mean = jnp.mean(train, axis=0)
        std = jnp.std(train, axis=0)
        train_s = (train - mean) / std
        return train_s, mean, std

    with jax.default_device(cpu):
        keys = jax.random.split(jax.random.key(42), B)
        train_s, mean, std = jax.vmap(sample_part)(jnp.asarray(images_np), keys)
        return (np.asarray(train_s), np.asarray(mean), np.asarray(std))


def _build_weights(train_s, mean, std):
    """Fold standardization + bias into W [B, 6, 100] fp32 (f64 math)."""
    s = train_s.astype(np.float64)          # [B,100,5]
    mn = mean.astype(np.float64)            # [B,5]
    sd = std.astype(np.float64)             # [B,5]
    a = s / sd[:, None, :]                  # [B,100,5]
    Wm = np.empty((B, 6, N_CAND), np.float64)
    Wm[:, 0:3, :] = (2.0 * a[:, :, 0:3] / COLOR_SCALE).transpose(0, 2, 1)
    Wm[:, 3:5, :] = (2.0 * a[:, :, 3:5]).transpose(0, 2, 1)
    Wm[:, 5, :] = -np.sum(s * s, axis=2) - 2.0 * np.sum(mn[:, None, :] * a, axis=2)
    return Wm.astype(np.float32)


def _pos_features():
    ii, jj = np.meshgrid(np.arange(H), np.arange(W), indexing="ij")
    pos = np.stack([ii, jj], -1).reshape(-1, 2).astype(np.float32)
    return pos / np.array([H, W], np.float32) * np.float32(POS_WEIGHT)  # [N_PIX,2]


def _build_bass():
    import concourse.bacc as bacc
    import concourse.mybir as mybir
    from concourse import tile

    f32 = mybir.dt.float32
    f16 = mybir.dt.float16
    nc = bacc.Bacc("TRN2", target_bir_lowering=False, debug=False)

    # fp16 hi/lo split folded into ONE K=24 matmul per tile:
    #   lhsT rows = [fh(6); fl(6); fh(6); fl(6)]
    #   rhs  rows = [Wh(6); Wh(6); Wl(6); Wl(6)]
    # so one MM accumulates fh.Wh + fl.Wh + fh.Wl + fl.Wl — the full
    # fp32-accurate product. MM time is N cycles regardless of K.
    f24_d = nc.dram_tensor("feats24", [IPC, 24, N_PIX], f16, kind="ExternalInput")
    w24_d = nc.dram_tensor("w24", [24, IPC, N_CAND], f16, kind="ExternalInput")
    imgpm_d = nc.dram_tensor("imgpm", [IPC, 128, NT * C], f32, kind="ExternalInput")
    out_d = nc.dram_tensor("out", [IPC, 128, NT * C], f32, kind="ExternalOutput")

    n_groups = (NT + GROUP - 1) // GROUP

    with tile.TileContext(nc) as tc:
        with (
            tc.tile_pool(name="const", bufs=1) as cpool,
            tc.tile_pool(name="sb", bufs=2) as sb,
            tc.tile_pool(name="scores", bufs=3) as scp,
            tc.tile_pool(name="psum", bufs=6, space="PSUM") as pp,
        ):
            # w24 rides the gpsimd queue so it lands concurrently with the
            # first feats chunk on the sync queue.
            w24_sb = cpool.tile([24, IPC, N_CAND], f16)
            nc.gpsimd.dma_start(out=w24_sb[:], in_=w24_d[:])

            # feats arrive in chunk-tiles (2/6/7 groups) so group 0's matmuls
            # start as soon as the first ~60KB land; imgpm/out ride the
            # gpsimd DMA queue to keep the sync queue purely feats.
            bounds = [0, 2 * GROUP, 8 * GROUP, 15 * GROUP]  # tiles [0,10,40,75]
            for i in range(IPC):
                f24_ch = []
                for ci in range(3):
                    t0, t1 = bounds[ci], min(bounds[ci + 1], NT)
                    ch = sb.tile(
                        [24, (t1 - t0) * 128], f16, tag=f"f24_{ci}", name=f"f24_{ci}"
                    )
                    nc.sync.dma_start(
                        out=ch[:], in_=f24_d[i, :, t0 * 128 : t1 * 128]
                    )
                    f24_ch.append(ch)
                imgpm_sb = sb.tile([128, NT * C], f32, tag="imgpm")
                nc.gpsimd.dma_start(out=imgpm_sb[:], in_=imgpm_d[i])

                # seg/mask runs in two halves (tiles 0..39 and 40..71) so the
                # first half's epilogue overlaps the second half's max8 stream;
                # split tiles keep the dependencies precise.
                HALF_T = [0, 8 * GROUP, NT]  # [0, 40, 72]
                fgmax_h = []
                bgmax_h = []
                for h in range(2):
                    nt_h = HALF_T[h + 1] - HALF_T[h]
                    fgmax_h.append(
                        sb.tile([128, nt_h * 8], f32, tag=f"fgmax{h}", name=f"fgmax{h}")
                    )
                    bgmax_h.append(
                        sb.tile([128, nt_h * 8], f32, tag=f"bgmax{h}", name=f"bgmax{h}")
                    )

                def emit_seg_half(h):
                    t0, t1 = HALF_T[h], HALF_T[h + 1]
                    nt_h = t1 - t0
                    seg = sb.tile([128, nt_h], f32, tag=f"seg{h}", name=f"seg{h}")
                    fg_r = fgmax_h[h][:].rearrange("p (t k) -> p t k", k=8)
                    bg_r = bgmax_h[h][:].rearrange("p (t k) -> p t k", k=8)
                    nc.vector.tensor_tensor(
                        seg[:], fg_r[:, :, 1], bg_r[:, :, 3], mybir.AluOpType.is_ge
                    )
                    out_sb = sb.tile(
                        [128, nt_h * C], f32, tag=f"out{h}", name=f"out{h}"
                    )
                    img_r = imgpm_sb[:, t0 * C : t1 * C].rearrange(
                        "p (t c) -> p t c", c=C
                    )
                    out_r = out_sb[:].rearrange("p (t c) -> p t c", c=C)
                    nc.vector.tensor_tensor(
                        out_r, img_r,
                        seg[:, :, None].to_broadcast([128, nt_h, C]),
                        mybir.AluOpType.mult,
                    )
                    nc.gpsimd.dma_start(
                        out=out_d[i, :, t0 * C : t1 * C], in_=out_sb[:]
                    )

                # 1-tile first group starts the max8 stream earliest;
                # 1-tile last group shortens the final epilogue chain.
                group_tiles = [1] + [GROUP] * ((NT - 2) // GROUP) + [1]
                t0g = 0
                for ntg in group_tiles:
                    ps = pp.tile([128, GROUP * N_CAND], f32, tag="ps")
                    for q in range(ntg):
                        t = t0g + q
                        ci = next(k for k in range(3) if t < bounds[k + 1])
                        off = (t - bounds[ci]) * 128
                        nc.tensor.matmul(
                            ps[:, q * N_CAND : (q + 1) * N_CAND],
                            f24_ch[ci][:, off : off + 128],
                            w24_sb[:, i, :],
                            start=True,
                            stop=True,
                        )
                    sc = scp.tile([128, GROUP * N_CAND], f32, tag="sc")
                    nc.scalar.copy(
                        out=sc[:, : ntg * N_CAND], in_=ps[:, : ntg * N_CAND]
                    )
                    for q in range(ntg):
                        t = t0g + q
                        h = 0 if t < HALF_T[1] else 1
                        toff = t - HALF_T[h]
                        nc.vector.max(
                            out=fgmax_h[h][:, toff * 8 : (toff + 1) * 8],
                            in_=sc[:, q * N_CAND : q * N_CAND + NPC],
                        )
                        nc.vector.max(
                            out=bgmax_h[h][:, toff * 8 : (toff + 1) * 8],
                            in_=sc[:, q * N_CAND + NPC : (q + 1) * N_CAND],
                        )
                    t0g += ntg
                    if t0g - ntg < HALF_T[1] <= t0g:
                        emit_seg_half(0)
                emit_seg_half(1)

    nc.compile()
    return nc


def _get_nc():
    if "nc" not in _CACHE:
        _CACHE["nc"] = _build_bass()
    return _CACHE["nc"]


def prepare_in_maps(images: np.ndarray) -> list:
    """Host preamble: sampling + weight folding + device data layouts."""
    images = np.asarray(images, dtype=np.float32)
    assert images.shape == (B, H, W, C)

    train_s, mean, std = _host_sampling(images)
    Wall = _build_weights(train_s, mean, std)        # [B,6,100]
    pos = _pos_features()                            # [N_PIX,2]

    flat = images.reshape(B, N_PIX, C)
    feats = np.empty((B, 6, N_PIX), np.float32)
    feats[:, 0:3, :] = flat.transpose(0, 2, 1)
    feats[:, 3:5, :] = pos.T[None]
    feats[:, 5, :] = 1.0
    f16h = feats.astype(np.float16)
    f16l = (feats - f16h.astype(np.float32)).astype(np.float16)
    w16h = Wall.astype(np.float16)
    w16l = (Wall - w16h.astype(np.float32)).astype(np.float16)
    # K=24 stacking: feats rows [fh; fl; fh; fl], W rows [Wh; Wh; Wl; Wl]
    f24 = np.concatenate([f16h, f16l, f16h, f16l], axis=1)          # [B,24,NPIX]
    w24 = np.concatenate([w16h, w16h, w16l, w16l], axis=1)          # [B,24,100]
    # pixel-major tiles of the PREPROCESSED image (img_p = img/255, which
    # equals the reference's clip(img/255,0,1) bit-exactly for inputs < 255):
    # imgpm[b, p, t*3+c] = img_p[b, t*128+p, c]. The device multiplies by the
    # 1.0/0.0 seg mask directly, so kept pixels are bit-exact.
    imgpm = np.ascontiguousarray(
        (flat / np.float32(255.0)).reshape(B, NT, 128, C).transpose(0, 2, 1, 3)
    ).reshape(B, 128, NT * C)

    in_maps = []
    for c in range(N_CORES):
        sl = slice(c * IPC, (c + 1) * IPC)
        in_maps.append(
            {
                "feats24": np.ascontiguousarray(f24[sl]),
                "w24": np.ascontiguousarray(w24[sl].transpose(1, 0, 2)),
                "imgpm": np.ascontiguousarray(imgpm[sl]),
            }
        )
    return in_maps


def assemble_output(results: list) -> np.ndarray:
    out = np.empty((B, N_PIX, C), np.float32)
    for c in range(N_CORES):
        o = results[c]["out"]  # [IPC, 128, NT*C]
        o = o.reshape(IPC, 128, NT, C).transpose(0, 2, 1, 3).reshape(IPC, N_PIX, C)
        out[c * IPC : (c + 1) * IPC] = o
    return out.reshape(B, H, W, C)


def kernel(images: np.ndarray) -> np.ndarray:
    from concourse.bass_utils import run_bass_kernel_spmd

    in_maps = prepare_in_maps(images)
    nc = _get_nc()
    res = run_bass_kernel_spmd(nc, in_maps, core_ids=list(range(N_CORES)))
    return assemble_output(res.results)
